# revision 23
# baseline (speedup 1.0000x reference)
"""Trainium2 Bass kernel for nn_DSSMEmbed (vq_codebook).

Strategy (8 NeuronCores, data-parallel over batch B=8192, Bc=1024/core),
single fused launch:

The index->embedding->conv_embed->conv1 chain is linear in the one-hot
encoding of s (14 dictionary entries x 25 pixels = 350 features), so it is
folded on the host into a single dense [350, 400] matrix per phi branch
(A1 for phi1 on s; A1d for phi2 on onehot(s')-onehot(s); biases folded too).
conv2 and the linear layer are dense matmuls as well ([400,800], [800,256]).
Everything on device is feature-major [features(partitions), batch(free)].

Per core: build one-hots via DMA-replicate + is_equal, run phi2 (fp32 -
feeds an argmax whose top-2 gaps reach 6e-6, so it needs full precision),
codebook scores + per-row argmax via DVE max/max_index, gather chosen zn
rows via indirect DMA, transpose to feature-major bf16 zmT [256,1024].
AllGather (bf16) the zmT blocks across the 8 cores through a DRAM bounce.
While the collective runs, compute phi1 (fp32r), normalize e1 with a
matmul-broadcast of exp(scale)/(||e1||+eps), cast to bf16. Finally the
gramm block [1024, 8192] = e1b.T @ zmT_full as bf16 matmuls, written to
HBM as bf16 (halves the write traffic; host upcasts to fp32).
"""
import sys
import numpy as np

try:
    import concourse.bass as bass
except ImportError:
    sys.path.insert(0, "/opt/trn_rl_repo")
    import concourse.bass as bass
import concourse.mybir as mybir
import concourse.tile as tile
from concourse import bacc
from concourse.bass_utils import run_bass_kernel_spmd
from concourse.masks import make_identity

F32 = mybir.dt.float32
F32R = mybir.dt.float32r
BF16 = mybir.dt.bfloat16
I32 = mybir.dt.int32
U32 = mybir.dt.uint32
AF = mybir.ActivationFunctionType

NCORES = 8
B, P, DICT = 8192, 25, 14
BC = B // NCORES          # 1024 per core
EPS = 1e-4

OH_CHUNKS = [(0, 125), (125, 250), (250, 350)]
F1_CHUNKS = [(0, 128), (128, 256), (256, 384), (384, 400)]
F2_CHUNKS = [(i * 128, min(800, (i + 1) * 128)) for i in range(7)]
E_CHUNKS = [(0, 128), (128, 256)]

# HW-probed (issue-gap) matmul rates at N=512, full clock: bf16 227ns,
# fp32 == fp32r 429ns. So bf16 is the only fast dtype; phi1/gramm are
# output-linear -> bf16 OK (rel gate 2e-2, bf16 chain err ~3e-3).
PHI1_DT = BF16
# phi2 branch feeds an argmax with top-2 gaps down to 6e-6 on this data;
# it must stay true fp32.
PHI2_DT = F32


# ---------------------------------------------------------------- host consts
def _tap(po, pi):
    oy, ox = divmod(po, 5)
    iy, ix = divmod(pi, 5)
    dy, dx = iy - oy + 1, ix - ox + 1
    return (dy, dx) if (0 <= dy < 3 and 0 <= dx < 3) else None


def _conv_as_matrix(w):
    O, C = w.shape[0], w.shape[1]
    M = np.zeros((C * P, O * P), np.float64)
    for po in range(P):
        for pi in range(P):
            t = _tap(po, pi)
            if t is None:
                continue
            dy, dx = t
            M[pi::P, po::P] += w[:, :, dy, dx].T.astype(np.float64)
    return M


def build_consts(i):
    t = i['embed_table'].astype(np.float64)
    n = np.sqrt((t * t).sum(1, keepdims=True))
    table_renorm = t * np.minimum(1.0, 1.0 / (n + 1e-7))

    w_e = i['conv_embed_w'].astype(np.float64)
    M9 = np.einsum('dc,ocyx->yxdo', table_renorm, w_e)
    T_emb = np.zeros((DICT * P, 64 * P))
    for po in range(P):
        for pi in range(P):
            tap = _tap(po, pi)
            if tap is None:
                continue
            T_emb[pi::P, po::P] += M9[tap[0], tap[1]]

    T_c1_1 = _conv_as_matrix(i['phi1_conv1_w'])
    T_c1_2 = _conv_as_matrix(i['phi2_conv1_w'])
    A1 = (T_emb @ T_c1_1).astype(np.float32)
    A1d = (T_emb @ T_c1_2).astype(np.float32)

    ce_b = i['conv_embed_b'].astype(np.float64)
    bias_map = np.repeat(ce_b[:, None], P, axis=1).reshape(-1)
    b1_eff = (bias_map @ T_c1_1
              + np.repeat(i['phi1_conv1_b'].astype(np.float64), P)).astype(np.float32)
    b2_eff = np.repeat(i['phi2_conv1_b'], P).astype(np.float32)

    A2 = _conv_as_matrix(i['phi1_conv2_w']).astype(np.float32)
    A2d = _conv_as_matrix(i['phi2_conv2_w']).astype(np.float32)
    b2x_1 = np.repeat(i['phi1_conv2_b'], P).astype(np.float32)
    b2x_2 = np.repeat(i['phi2_conv2_b'], P).astype(np.float32)

    lwT1 = np.ascontiguousarray(i['phi1_lin_w'].T).astype(np.float32)
    lwT2 = np.ascontiguousarray(i['phi2_lin_w'].T).astype(np.float32)

    z = i['z_vectors'].astype(np.float64)
    zn = (z / np.sqrt((z * z).sum(1, keepdims=True))).astype(np.float32)
    znT = np.ascontiguousarray(zn.T)

    exp_scale = float(np.exp(np.float64(i['scale'][0])))

    def pad_pk(m, pk=128):
        out = np.zeros((pk, m.shape[1]), np.float32)
        out[:m.shape[0]] = m
        return out

    c = {}
    # K-chunked lhsT matrices, padded to 128 partitions
    c['a1'] = [pad_pk(A1[s0:s1]) for s0, s1 in OH_CHUNKS]         # 3 x [128,400]
    c['a1d'] = [pad_pk(A1d[s0:s1]) for s0, s1 in OH_CHUNKS]
    c['a2'] = [pad_pk(A2[s0:s1]) for s0, s1 in F1_CHUNKS]         # 4 x [128,800]
    c['a2d'] = [pad_pk(A2d[s0:s1]) for s0, s1 in F1_CHUNKS]
    c['lw1'] = [pad_pk(lwT1[s0:s1]) for s0, s1 in F2_CHUNKS]      # 7 x [128,256]
    c['lw2'] = [pad_pk(lwT2[s0:s1]) for s0, s1 in F2_CHUNKS]
    c['znt'] = [np.ascontiguousarray(znT[s0:s1]) for s0, s1 in E_CHUNKS]  # 2x[128,512]
    c['zn'] = zn                                                   # [512,256] gather src

    def colpack(v, chunks, pk=128):
        # [F] vector -> [128, nchunks] column-per-chunk
        out = np.zeros((pk, len(chunks)), np.float32)
        for j, (s0, s1) in enumerate(chunks):
            out[:s1 - s0, j] = v[s0:s1]
        return out

    c['b1c'] = colpack(b1_eff, F1_CHUNKS)
    c['b2c'] = colpack(b2_eff, F1_CHUNKS)
    c['b2x1c'] = colpack(b2x_1, F2_CHUNKS)
    c['b2x2c'] = colpack(b2x_2, F2_CHUNKS)
    c['lb1c'] = colpack(i['phi1_lin_b'], E_CHUNKS)
    c['lb2c'] = colpack(i['phi2_lin_b'], E_CHUNKS)
    # d-iota per OH chunk (value = d of that partition), packed as columns
    io = np.zeros((128, 3), np.float32)
    for j, (s0, s1) in enumerate(OH_CHUNKS):
        io[:s1 - s0, j] = (np.arange(s0, s1) // P).astype(np.float32)
        io[s1 - s0:, j] = -1.0  # never equal to s values
    c['iotac'] = io
    c['expsc'] = np.full((128, 1), exp_scale, np.float32)
    rm = np.zeros((128, 125), np.float32)
    for dd in range(5):
        rm[np.arange(P), dd * P + np.arange(P)] = 1.0
    c['repm'] = rm
    return c


# ---------------------------------------------------------------- fused IR
def build_fused():
    nc = bacc.Bacc("TRN2", target_bir_lowering=False, debug=False)
    din = {}
    din['sT'] = nc.dram_tensor("sT", [P, BC], BF16, kind="ExternalInput")
    din['spT'] = nc.dram_tensor("spT", [P, BC], BF16, kind="ExternalInput")
    for name, shape in [
        ("a1_0", [128, 400]), ("a1_1", [128, 400]), ("a1_2", [128, 400]),
        ("a1d_0", [128, 400]), ("a1d_1", [128, 400]), ("a1d_2", [128, 400]),
        ("a2_0", [128, 800]), ("a2_1", [128, 800]), ("a2_2", [128, 800]),
        ("a2_3", [128, 800]),
        ("a2d_0", [128, 800]), ("a2d_1", [128, 800]), ("a2d_2", [128, 800]),
        ("a2d_3", [128, 800]),
    ]:
        w_dt = PHI1_DT if name.startswith(("a1_", "a2_")) else F32
        din[name] = nc.dram_tensor(name, shape, w_dt, kind="ExternalInput")
    for j in range(7):
        din[f"lw1_{j}"] = nc.dram_tensor(f"lw1_{j}", [128, 256], PHI1_DT, kind="ExternalInput")
        din[f"lw2_{j}"] = nc.dram_tensor(f"lw2_{j}", [128, 256], F32, kind="ExternalInput")
    din['znt_0'] = nc.dram_tensor("znt_0", [128, 512], F32, kind="ExternalInput")
    din['znt_1'] = nc.dram_tensor("znt_1", [128, 512], F32, kind="ExternalInput")
    din['zn'] = nc.dram_tensor("zn", [512, 256], BF16, kind="ExternalInput")
    din['b1c'] = nc.dram_tensor("b1c", [128, 4], F32, kind="ExternalInput")
    din['b2c'] = nc.dram_tensor("b2c", [128, 4], F32, kind="ExternalInput")
    din['b2x1c'] = nc.dram_tensor("b2x1c", [128, 7], F32, kind="ExternalInput")
    din['b2x2c'] = nc.dram_tensor("b2x2c", [128, 7], F32, kind="ExternalInput")
    din['lb1c'] = nc.dram_tensor("lb1c", [128, 2], F32, kind="ExternalInput")
    din['lb2c'] = nc.dram_tensor("lb2c", [128, 2], F32, kind="ExternalInput")
    din['iotac'] = nc.dram_tensor("iotac", [128, 3], F32, kind="ExternalInput")
    din['repm'] = nc.dram_tensor("repm", [128, 125], BF16, kind="ExternalInput")
    din['expsc'] = nc.dram_tensor("expsc", [128, 1], F32, kind="ExternalInput")

    gout = nc.dram_tensor("gramm", [BC, B], BF16, kind="ExternalOutput")

    NT = BC // 512  # N tiles of 512

    with tile.TileContext(nc) as tc:
        with (
            tc.tile_pool(name="wpool", bufs=1) as wp,
            tc.tile_pool(name="act", bufs=1) as ap,
            tc.tile_pool(name="scr", bufs=2) as scr,
            tc.tile_pool(name="zr", bufs=2) as zrp,
            tc.tile_pool(name="got", bufs=4) as gop,
            tc.tile_pool(name="ps", bufs=2, space="PSUM") as ps,
            tc.tile_pool(name="ps1", bufs=1, space="PSUM") as ps1,
            tc.tile_pool(name="psg", bufs=4, space="PSUM") as psg,  # 2+1+1+4 = 8 banks
            tc.tile_pool(name="dram", bufs=1, space="DRAM") as dp,
        ):
            # ---- PE warm-up: ~40 dummy accumulating matmuls on the identity
            # tile (no DMA deps) so the HAM clock gate opens to 8/8 while the
            # first weight DMAs are still streaming
            ident = scr.tile([128, 128], BF16, tag="ident", bufs=1)
            make_identity(nc, ident[:])
            pwu = ps.tile([128, 512], F32, tag="mm", name="pwu")
            for r in range(40):
                nc.tensor.matmul(pwu[:, 0:128], ident[:], ident[:],
                                 start=(r == 0), stop=(r == 39))
            wud = scr.tile([128, 128], BF16, tag="wud", bufs=1)
            nc.vector.tensor_copy(wud[:], pwu[:, 0:128])

            # ---- load constants; order = consumption order so the first
            # matmuls (one-hot replicate, phi2 layer 1) start ASAP
            load_order = (["repm", "iotac", "b2c", "a1d_0", "a1d_1", "a1d_2",
                           "b2x2c", "a2d_0", "a2d_1", "a2d_2", "a2d_3",
                           "lb2c"] + [f"lw2_{j}" for j in range(7)]
                          + ["znt_0", "znt_1", "b1c", "a1_0", "a1_1", "a1_2",
                             "b2x1c", "a2_0", "a2_1", "a2_2", "a2_3", "lb1c"]
                          + [f"lw1_{j}" for j in range(7)] + ["expsc"])
            W = {}

            def loadc(name):
                th = din[name]
                t = wp.tile(list(th.shape), th.dtype, tag=name, name=name)
                nc.sync.dma_start(t[:], th[:])
                W[name] = t

            # inputs + one-hot consts first: the replicate matmuls and phi2
            # layer 1 are the head of the critical path
            loadc("repm")
            loadc("iotac")
            ts = ap.tile([P, BC], BF16, tag="ts")
            tsp = ap.tile([P, BC], BF16, tag="tsp")
            nc.sync.dma_start(ts[:], din['sT'][:])
            nc.sync.dma_start(tsp[:], din['spT'][:])
            for name in load_order:
                if name not in W:
                    loadc(name)

            # one-hots via PE replicate-matmul (rep = repm.T@ts stacks ts
            # nd times along partitions), then DVE is_equal against the d-iota
            oh, ohd = [], []
            for kc, (s0, s1) in enumerate(OH_CHUNKS):
                kw = s1 - s0
                t_oh = ap.tile([128, BC], PHI1_DT, tag=f"oh{kc}", name=f"oh{kc}")
                t_ohd = ap.tile([128, BC], F32, tag=f"ohd{kc}", name=f"ohd{kc}")
                iot = W['iotac'][:, kc:kc + 1]
                for n in range(NT):
                    nsl = slice(n * 512, (n + 1) * 512)
                    pr_s = ps.tile([128, 512], F32, tag="mm", name="pr_s")
                    nc.tensor.matmul(pr_s[:kw, :], W['repm'][:P, :kw],
                                     ts[:, nsl],
                                     start=True, stop=True)
                    pr_p = ps.tile([128, 512], F32, tag="mm", name="pr_p")
                    nc.tensor.matmul(pr_p[:kw, :], W['repm'][:P, :kw],
                                     tsp[:, nsl],
                                     start=True, stop=True)
                    nc.vector.tensor_scalar(t_oh[:kw, nsl], pr_s[:kw, :],
                                            iot[:kw], None,
                                            mybir.AluOpType.is_equal)
                    nc.vector.tensor_scalar(t_ohd[:kw, nsl], pr_p[:kw, :],
                                            iot[:kw], None,
                                            mybir.AluOpType.is_equal)
                    nc.vector.tensor_tensor(t_ohd[:kw, nsl], t_ohd[:kw, nsl],
                                            t_oh[:kw, nsl],
                                            op=mybir.AluOpType.subtract)
                oh.append(t_oh)
                ohd.append(t_ohd)

            def chain_mm(rhs_tiles, rhs_chunks, lhs_names, m_chunks, nt, dt,
                         out_tag, bias_col=None, relu=False, out_dt=F32,
                         n_range=None, outs=None, m_range=None):
                """out[m][:, n*512...] = act(sum_k lhsT_k[:,mslice].T @ rhs_k[:,nslice])."""
                if outs is None:
                    outs = [ap.tile([128, BC], out_dt, tag=f"{out_tag}{mi}",
                                    name=f"{out_tag}{mi}")
                            for mi in range(len(m_chunks))]
                for mi in (m_range if m_range is not None
                           else range(len(m_chunks))):
                    m0, m1 = m_chunks[mi]
                    mw = m1 - m0
                    o = outs[mi]
                    for n in (n_range if n_range is not None else range(nt)):
                        nsl = slice(n * 512, (n + 1) * 512)
                        pt = ps.tile([128, 512], F32, tag="mm")
                        nk = len(lhs_names)
                        for k in range(nk):
                            kw = rhs_chunks[k][1] - rhs_chunks[k][0]
                            nc.tensor.matmul(
                                pt[:mw, :],
                                W[lhs_names[k]][:kw, m0:m1],
                                rhs_tiles[k][:kw, nsl],
                                start=(k == 0), stop=(k == nk - 1))
                        if bias_col is not None:
                            bc = W[bias_col][:, mi:mi + 1]
                            nc.scalar.activation(o[:mw, nsl], pt[:mw, :],
                                                 AF.Relu if relu else AF.Identity,
                                                 bias=bc[:mw])
                        else:
                            nc.scalar.activation(o[:mw, nsl], pt[:mw, :],
                                                 AF.Relu if relu else AF.Copy)
                return outs

            # ---- phi2 branch first (fp32): feeds argmax -> zmT -> collective
            x1d = chain_mm(ohd, OH_CHUNKS, ["a1d_0", "a1d_1", "a1d_2"], F1_CHUNKS,
                           NT, PHI2_DT, "x1", bias_col="b2c", relu=True)
            x2d = chain_mm(x1d, F1_CHUNKS, ["a2d_0", "a2d_1", "a2d_2", "a2d_3"],
                           F2_CHUNKS, NT, PHI2_DT, "x2", bias_col="b2x2c", relu=True)
            # ---- phi2 linear, then scores + argmax + gather + transpose per
            # 128-batch block; the two AllGather halves are emitted after the
            # loop but their input DMAs fire mid-loop via subtile deps (half 0
            # as soon as blocks 0-3 have landed in zmb).
            zmb = [ap.tile([128, BC], BF16, tag=f"zmb{k}", name=f"zmb{k}")
                   for k in range(2)]
            HB = BC // 2
            e2 = chain_mm(x2d, F2_CHUNKS, [f"lw2_{j}" for j in range(7)],
                          E_CHUNKS, NT, PHI2_DT, "e2", bias_col="lb2c",
                          relu=False)
            for bi in range(BC // 128):
                bsl = slice(bi * 128, (bi + 1) * 128)
                psc = ps.tile([128, 512], F32, tag="mm", name="psc")
                for k in range(2):
                    nc.tensor.matmul(psc[:], e2[k][:, bsl], W[f'znt_{k}'][:],
                                     start=(k == 0), stop=(k == 1))
                mx = scr.tile([128, 8], F32, tag="mx")
                mi_ = scr.tile([128, 8], U32, tag="mi")
                nc.vector.max(mx[:], psc[:])
                nc.vector.max_index(mi_[:], mx[:], psc[:])
                gi = scr.tile([128, 1], I32, tag="gi")
                nc.vector.tensor_copy(gi[:], mi_[:, 0:1].bitcast(I32))
                zg = scr.tile([128, 256], BF16, tag="zg")
                nc.gpsimd.indirect_dma_start(
                    out=zg[:], out_offset=None, in_=din['zn'][:],
                    in_offset=bass.IndirectOffsetOnAxis(ap=gi[:, 0:1], axis=0))
                for k in range(2):
                    ptr = ps.tile([128, 128], BF16, tag="ptr", bufs=1)
                    nc.tensor.transpose(ptr[:], zg[:, k * 128:(k + 1) * 128],
                                        ident[:])
                    nc.vector.tensor_copy(zmb[k][:, bsl], ptr[:])

            ccout_h = []
            for h in range(2):
                cci = dp.tile([256, HB], BF16, tag=f"ccin{h}", name=f"ccin{h}")
                for k in range(2):
                    nc.sync.dma_start(cci[k * 128:(k + 1) * 128, :],
                                      zmb[k][:, h * HB:(h + 1) * HB])
                cco = dp.tile([NCORES * 256, HB], BF16, tag=f"ccout{h}",
                              name=f"ccout{h}", addr_space="Shared")
                nc.gpsimd.collective_compute(
                    "AllGather", mybir.AluOpType.bypass,
                    replica_groups=[list(range(NCORES))],
                    ins=[cci[:].opt()], outs=[cco[:].opt()])
                ccout_h.append(cco)

            # ---- phi1 branch (bf16), overlaps the collectives
            x1 = chain_mm(oh, OH_CHUNKS, ["a1_0", "a1_1", "a1_2"], F1_CHUNKS,
                          NT, PHI1_DT, "x1", bias_col="b1c", relu=True,
                          out_dt=PHI1_DT)
            x2 = chain_mm(x1, F1_CHUNKS, ["a2_0", "a2_1", "a2_2", "a2_3"],
                          F2_CHUNKS, NT, PHI1_DT, "x2", bias_col="b2x1c", relu=True,
                          out_dt=PHI1_DT)
            e1 = chain_mm(x2, F2_CHUNKS, [f"lw1_{j}" for j in range(7)],
                          E_CHUNKS, NT, PHI1_DT, "e1", bias_col="lb1c", relu=False)

            # ---- e1 normalization: r = exp(scale)/(sqrt(sum e1^2)+eps).
            # Broadcast the row-sums to 128 partitions via a K=1 matmul FIRST,
            # then run sqrt/eps/recip on fat [128,512] tiles (short critical
            # chain; the PE moves on to gramm as soon as e1b halves appear).
            ones = scr.tile([128, 1], F32, tag="ones", bufs=1)
            nc.gpsimd.memset(ones[:], 1.0)
            onesr = scr.tile([1, 128], F32, tag="onesr", bufs=1)
            nc.gpsimd.memset(onesr[:], 1.0)
            e1sq = ap.tile([128, BC], F32, tag="e1sq")
            nrow = scr.tile([1, BC], F32, tag="nrow", bufs=1)
            e1b = [ap.tile([128, BC], BF16, tag=f"e1b{k}", name=f"e1b{k}")
                   for k in range(2)]
            for n in range(NT):
                nsl = slice(n * 512, (n + 1) * 512)
                pn = ps1.tile([128, 512], F32, tag="pn", name="pn")[0:1, :]
                for k in range(2):
                    nc.vector.tensor_tensor(e1sq[:, nsl], e1[k][:, nsl],
                                            e1[k][:, nsl],
                                            op=mybir.AluOpType.mult)
                    nc.tensor.matmul(pn[:, :], ones[:], e1sq[:, nsl],
                                     start=(k == 0), stop=(k == 1))
                nc.vector.tensor_copy(nrow[:, nsl], pn[:, :])
                pr = psg.tile([128, 512], F32, tag="gmm", name="prb")
                nc.tensor.matmul(pr[:], onesr[:], nrow[:, nsl],
                                 start=True, stop=True)
                sq = scr.tile([128, 512], F32, tag="sq")
                nc.scalar.activation(sq[:], pr[:], AF.Sqrt)
                nc.vector.tensor_scalar_add(sq[:], sq[:], EPS)
                nc.vector.reciprocal(sq[:], sq[:])
                nc.vector.tensor_scalar(sq[:], sq[:], W['expsc'][:, 0:1], None,
                                        mybir.AluOpType.mult)
                for k in range(2):
                    nc.vector.tensor_tensor(e1b[k][:, nsl], e1[k][:, nsl],
                                            sq[:], op=mybir.AluOpType.mult)

            # ---- gramm block [BC, B] = e1b.T @ zmT_full, bf16.
            # Column-slot layout (host permutes): slot 0 = the LOCAL block
            # (cb == pid, straight from SBUF zmb -> runs during the
            # collectives); slot 1+i = cb (pid+1+i) & 7, read from the
            # gathered halves at a dynamic row offset. All gout writes are
            # static. nj (= collective half) is the outer loop.
            pid_g = nc.gpsimd.partition_id()
            pid_s = nc.scalar.partition_id()
            for nj in range(2):
                for mi in range(BC // 128):
                    msl = slice(mi * 128, (mi + 1) * 128)
                    pt = psg.tile([128, 512], F32, tag="gmm", name="ptl")
                    for k in range(2):
                        nc.tensor.matmul(
                            pt[:], e1b[k][:, msl],
                            zmb[k][:, nj * 512:(nj + 1) * 512],
                            start=(k == 0), stop=(k == 1))
                    ot = gop.tile([128, 512], BF16, tag="ot", name="otl")
                    nc.any.tensor_copy(ot[:], pt[:])
                    nc.sync.dma_start(gout[msl, nj * 512:(nj + 1) * 512],
                                      ot[:])
                for i in range(NCORES - 1):
                    cbe_g = (pid_g + 1 + i) & 7
                    cbe_s = (pid_s + 1 + i) & 7
                    zr = [zrp.tile([128, HB], BF16, tag=f"zr{k}",
                                   name=f"zr{k}_{nj}_{i}", bufs=3)
                          for k in range(2)]
                    nc.gpsimd.dma_start(
                        zr[0][:],
                        ccout_h[nj][bass.ds(cbe_g * 256, 128), :])
                    nc.scalar.dma_start(
                        zr[1][:],
                        ccout_h[nj][bass.ds(cbe_s * 256 + 128, 128), :])
                    base = (1 + i) * BC + nj * 512
                    for mi in range(BC // 128):
                        msl = slice(mi * 128, (mi + 1) * 128)
                        pt = psg.tile([128, 512], F32, tag="gmm", name="ptr_")
                        for k in range(2):
                            nc.tensor.matmul(
                                pt[:], e1b[k][:, msl], zr[k][:],
                                start=(k == 0), stop=(k == 1))
                        ot = gop.tile([128, 512], BF16, tag="ot", name="otr")
                        nc.any.tensor_copy(ot[:], pt[:])
                        nc.sync.dma_start(gout[msl, base:base + 512], ot[:])
    nc.compile()
    return nc


# ---------------------------------------------------------------- entry point
_CACHE = {}


def _get_nc(key, builder):
    if key not in _CACHE:
        _CACHE[key] = builder()
    return _CACHE[key]


def build_in_maps(i):
    import ml_dtypes
    bf16 = ml_dtypes.bfloat16
    c = build_consts(i)
    s = i['s'].reshape(B, P).astype(np.float32)
    sp = i['s_prime'].reshape(B, P).astype(np.float32)

    const_map = {}
    bf16_names = {"a1", "a2", "lw1"}
    for pfx, arrs in [("a1", c['a1']), ("a1d", c['a1d']), ("a2", c['a2']),
                      ("a2d", c['a2d']), ("lw1", c['lw1']), ("lw2", c['lw2']),
                      ("znt", c['znt'])]:
        for j, a in enumerate(arrs):
            a = np.ascontiguousarray(a)
            if pfx in bf16_names:
                a = a.astype(bf16)
            const_map[f"{pfx}_{j}"] = a
    for name in ("b1c", "b2c", "b2x1c", "b2x2c", "lb1c", "lb2c", "iotac",
                 "expsc"):
        const_map[name] = c[name]
    const_map['repm'] = c['repm'].astype(bf16)
    const_map['zn'] = c['zn'].astype(bf16)

    in_maps = []
    for core in range(NCORES):
        sl = slice(core * BC, (core + 1) * BC)
        m = dict(const_map)
        m['sT'] = np.ascontiguousarray(s[sl].T).astype(bf16)
        m['spT'] = np.ascontiguousarray(sp[sl].T).astype(bf16)
        in_maps.append(m)
    return in_maps


def kernel(**inputs):
    i = {k: np.asarray(v) for k, v in inputs.items()}
    in_maps = build_in_maps(i)

    import time
    nc = _get_nc("fused", build_fused)
    t0 = time.time()
    res = run_bass_kernel_spmd(nc, in_maps, list(range(NCORES)))
    t1 = time.time()
    global LAST_WALL
    LAST_WALL = dict(fused=t1 - t0)

    out = np.empty((B, B), np.float32)
    for core in range(NCORES):
        g = res.results[core]['gramm']
        rows = slice(core * BC, (core + 1) * BC)
        for j in range(NCORES):
            cb = core if j == 0 else (core + j) % NCORES
            out[rows, cb * BC:(cb + 1) * BC] = g[:, j * BC:(j + 1) * BC]
    return out


LAST_WALL = None
